# revision 1
# baseline (speedup 1.0000x reference)
"""DocumentDualEmbedder pooling kernel for Trainium2 (Bass/Tile).

Computes, per document b:
    w   = idf[chunk[b]];  w_n = w / sum(w)
    out[b] = concat(sum_s w_n[s] * x[s],   # idf-weighted mean   [D]
                    max_s x[s],            # max pool            [D]
                    min_s x[s],            # min pool            [D]
                    sqrt(var_s x[s]))      # unbiased std        [D]

Full shapes: chunk [64, 2048] i32, encoding [64, 2048, 256] f32,
idf [32000] f32 -> out [64, 1024] f32.

Distribution: pure data parallel over the batch dim; each of the 8
NeuronCores processes 8 documents; idf is replicated. No collectives.

Per-core algorithm (BL=8 docs, S=2048, D=256):
  - seq position s is mapped to (partition q = s//16, chunk t = s%16) so
    each partition's load is one contiguous 16KB run of DRAM.
  - encoding is cast f32->bf16 during the DMA (SWDGE cast).
  - mean/mu/E[x^2] via PE: per (doc, chunk) matmul with stationary
    [w_norm | 1/S] (bf16) against rhs x_chunk / square(x_chunk),
    accumulated in separate PSUM banks -> two [2, 256] tiles per doc.
  - max/min via DVE tensor_tensor trees in bf16 (2x mode), then a gpsimd
    partition_all_reduce across partitions (min via -max(-x)).
  - std = sqrt(S/(S-1) * (E[x^2] - mu^2)) on ACT.
  - The partition phase + epilogue run per half (docs 0-3 / 4-7) so the
    tail overlaps the second half's main loop.
"""

import os
import numpy as np

import concourse.bass as bass
import concourse.bacc as bacc
import concourse.tile as tile
from concourse import mybir, bass_isa
from concourse.bass_utils import run_bass_kernel_spmd

# Problem constants (hardcoded; kernel.py must be self-contained).
B, S, D, V = 64, 2048, 256, 32000
NCORES = 8
BL = B // NCORES          # docs per core
T = 16                    # chunks per doc (s % 16)
P = 128                   # partitions (s // 16)
HB = BL // 2              # half-batch for tail pipelining
F32 = mybir.dt.float32
BF16 = mybir.dt.bfloat16
I32 = mybir.dt.int32

# If True, gather w=idf[chunk] on host and pass as an input (pre-arranged in
# the on-device layout).  Device-side scalar gather is limited to 128
# elements per indirect-DMA instruction on trn2, which makes the 16K-element
# gather prohibitively expensive on-device (~128 gpsimd instructions).
HOST_GATHER = os.environ.get("KERNEL_HOST_GATHER", "1") == "1"


def build_bass(reps: int = 1):
    nc = bacc.Bacc("TRN2", target_bir_lowering=False, debug=False)
    chunk_d = nc.dram_tensor("chunk", [BL, S], I32, kind="ExternalInput")
    enc_d = nc.dram_tensor("encoding", [BL, S, D], F32, kind="ExternalInput")
    idf_d = nc.dram_tensor("idf", [V, 1], F32, kind="ExternalInput")
    if HOST_GATHER:
        # w_arr[q, b*T + t] = idf[chunk[b, q*16 + t]]  (device layout)
        w_arr_d = nc.dram_tensor("w_arr", [P, BL * T], F32, kind="ExternalInput")
    out_d = nc.dram_tensor("out", [BL, 4 * D], F32, kind="ExternalOutput")

    with tile.TileContext(nc) as tc:
      for _rep in range(reps):
        with (
            tc.tile_pool(name="singles", bufs=1) as singles,
            tc.tile_pool(name="xpool", bufs=3) as xpool,
            tc.tile_pool(name="sqpool", bufs=2) as sqpool,
            tc.tile_pool(name="treepool", bufs=2) as treepool,
            tc.tile_pool(name="psum", bufs=4, space="PSUM") as psum,
        ):
            # ---------------- w preparation ----------------
            # w_dir[q, b*T+t] = idf[chunk[b, q*16+t]]
            w_dir = singles.tile([P, BL * T], F32)
            if HOST_GATHER:
                nc.sync.dma_start(out=w_dir[:], in_=w_arr_d[:, :])
            else:
                idx_sb = singles.tile([P, BL, T], I32)
                nc.sync.dma_start(
                    out=idx_sb[:],
                    in_=chunk_d[:, :].rearrange("b (q t) -> q b t", t=T))
                for c in range(BL * T):
                    nc.gpsimd.indirect_dma_start(
                        out=w_dir[:, c:c + 1],
                        out_offset=None,
                        in_=idf_d[:, :],
                        in_offset=bass.IndirectOffsetOnAxis(
                            ap=idx_sb[:, c // T, c % T:c % T + 1], axis=0),
                    )

            # Sum over all partitions (per column), then over t per doc.
            wsum = singles.tile([P, BL * T], F32)
            nc.gpsimd.partition_all_reduce(
                wsum[:], w_dir[:], channels=P, reduce_op=bass_isa.ReduceOp.add
            )
            swb = singles.tile([P, BL], F32)
            nc.vector.reduce_sum(
                swb[:], wsum[:].rearrange("q (b t) -> q b t", t=T),
                axis=mybir.AxisListType.X,
            )
            rinv = singles.tile([P, BL], F32)
            nc.vector.reciprocal(rinv[:], swb[:])

            # wl[q, b, t, :] = (w_norm, 1/S) in bf16 (stationary operands).
            wl = singles.tile([P, BL, T, 2], BF16)
            nc.vector.memset(wl[:], 1.0 / S)
            for b in range(BL):
                nc.vector.tensor_scalar_mul(
                    wl[:, b, :, 0],
                    w_dir[:, b * T:(b + 1) * T],
                    rinv[:, b:b + 1],
                )

            # ---------------- per-doc main loop ----------------
            # Per-half accumulators for max/min partition reduction.
            mall = [singles.tile([P, HB, D], BF16, name=f"mall{h}", tag=f"mall{h}")
                    for h in range(2)]
            nall = [singles.tile([P, HB, D], BF16, name=f"nall{h}", tag=f"nall{h}")
                    for h in range(2)]
            # PSUM row drains: [2, BL, 512] f32 (cols 0:256 from rhs=x:
            # [mean | mu], cols 256:512 from rhs=x^2: [junk | E]).
            stats_sb = singles.tile([2, BL, 2 * D], F32)

            def tail_half(h):
                """Partition reduction + epilogue for docs h*HB..h*HB+HB-1."""
                b0 = h * HB
                mred = singles.tile([P, HB, D], F32, tag=f"mred{h}")
                nc.gpsimd.partition_all_reduce(
                    mred[:], mall[h][:], channels=P,
                    reduce_op=bass_isa.ReduceOp.max)
                nneg = singles.tile([P, HB, D], BF16, tag=f"nneg{h}")
                nc.vector.tensor_scalar_mul(nneg[:], nall[h][:], -1.0)
                nred = singles.tile([P, HB, D], F32, tag=f"nred{h}")
                nc.gpsimd.partition_all_reduce(
                    nred[:], nneg[:], channels=P,
                    reduce_op=bass_isa.ReduceOp.max)
                minrow = singles.tile([1, HB, D], F32, tag=f"minrow{h}")
                nc.vector.tensor_scalar_mul(minrow[:], nred[0:1, :, :], -1.0)

                # Relocate per-doc stats rows to one partition per doc.
                strow = singles.tile([HB, 2, 2 * D], F32, tag=f"strow{h}")
                nc.sync.dma_start(out=strow[:, 0:1, :],
                                  in_=stats_sb[0:1, b0:b0 + HB, :])
                nc.sync.dma_start(out=strow[:, 1:2, :],
                                  in_=stats_sb[1:2, b0:b0 + HB, :])

                musq = singles.tile([HB, D], F32, tag=f"musq{h}")
                nc.vector.tensor_tensor(
                    musq[:], strow[:, 1, 0:D], strow[:, 1, 0:D],
                    op=mybir.AluOpType.mult,
                )
                var0 = singles.tile([HB, D], F32, tag=f"var0{h}")
                nc.vector.tensor_tensor(
                    var0[:], strow[:, 1, D:2 * D], musq[:],
                    op=mybir.AluOpType.subtract,
                )
                stdv = singles.tile([HB, D], F32, tag=f"stdv{h}")
                nc.scalar.activation(
                    stdv[:], var0[:], mybir.ActivationFunctionType.Sqrt,
                    scale=float(S) / float(S - 1),
                )

                osl = slice(b0, b0 + HB)
                nc.sync.dma_start(out=out_d[osl, 0:D], in_=strow[:, 0, 0:D])
                nc.sync.dma_start(out=out_d[osl, D:2 * D], in_=mred[0:1, :, :])
                nc.sync.dma_start(out=out_d[osl, 2 * D:3 * D], in_=minrow[:])
                nc.sync.dma_start(out=out_d[osl, 3 * D:4 * D], in_=stdv[:])

            for b in range(BL):
                h, bh = divmod(b, HB)
                x_b = xpool.tile([P, T, D], BF16, tag="x")
                # enc[b] as [q, (t d)]: per-partition contiguous 16KB run.
                nc.gpsimd.dma_start(
                    out=x_b[:],
                    in_=enc_d[b, :, :].rearrange("(q t) d -> q (t d)", t=T),
                )

                sq_b = sqpool.tile([P, T, D], BF16, tag="sq")
                nc.scalar.activation(
                    sq_b[:], x_b[:], mybir.ActivationFunctionType.Square
                )

                # PE: accumulate [w|1/S]^T @ x and @ x^2 over chunks.
                # Separate PSUM banks: interleaved accumulation groups in one
                # bank corrupt each other.
                ps_a = psum.tile([2, D], F32, tag="pa")
                ps_b = psum.tile([2, D], F32, tag="pb")
                for t in range(T):
                    nc.tensor.matmul(
                        ps_a[:],
                        lhsT=wl[:, b, t, :],
                        rhs=x_b[:, t, :],
                        start=(t == 0),
                        stop=(t == T - 1),
                    )
                    nc.tensor.matmul(
                        ps_b[:],
                        lhsT=wl[:, b, t, :],
                        rhs=sq_b[:, t, :],
                        start=(t == 0),
                        stop=(t == T - 1),
                    )
                # Drain PSUM -> SBUF; alternate DVE/ACT to balance engines.
                if b % 2 == 0:
                    nc.vector.tensor_copy(stats_sb[:, b, 0:D], ps_a[:])
                    nc.vector.tensor_copy(stats_sb[:, b, D:2 * D], ps_b[:])
                else:
                    nc.scalar.copy(stats_sb[:, b, 0:D], ps_a[:])
                    nc.scalar.copy(stats_sb[:, b, D:2 * D], ps_b[:])

                # DVE max/min trees over chunks: 16 -> 8 -> 4 -> 2 -> 1.
                for stat, alu, acc in (
                    ("mx", mybir.AluOpType.max, mall[h]),
                    ("mn", mybir.AluOpType.min, nall[h]),
                ):
                    t1 = treepool.tile([P, 8, D], BF16, tag=f"{stat}1")
                    nc.vector.tensor_tensor(
                        t1[:], x_b[:, 0:8, :], x_b[:, 8:16, :], op=alu
                    )
                    t2 = treepool.tile([P, 4, D], BF16, tag=f"{stat}2")
                    nc.vector.tensor_tensor(
                        t2[:], t1[:, 0:4, :], t1[:, 4:8, :], op=alu
                    )
                    t3 = treepool.tile([P, 2, D], BF16, tag=f"{stat}3")
                    nc.vector.tensor_tensor(
                        t3[:], t2[:, 0:2, :], t2[:, 2:4, :], op=alu
                    )
                    nc.vector.tensor_tensor(
                        acc[:, bh, :], t3[:, 0, :], t3[:, 1, :], op=alu
                    )

                if b == HB - 1:
                    tail_half(0)
            tail_half(1)

    nc.finalize()
    return nc


_NC = None


def _get_nc():
    global _NC
    if _NC is None:
        _NC = build_bass()
    return _NC


def make_in_maps(chunk, encoding, idf):
    chunk = np.ascontiguousarray(np.asarray(chunk, dtype=np.int32))
    encoding = np.ascontiguousarray(np.asarray(encoding, dtype=np.float32))
    idf = np.ascontiguousarray(np.asarray(idf, dtype=np.float32)).reshape(V, 1)
    in_maps = []
    for c in range(NCORES):
        chunk_c = chunk[c * BL:(c + 1) * BL]
        m = {
            "chunk": chunk_c,
            "encoding": encoding[c * BL:(c + 1) * BL],
            "idf": idf,
        }
        if HOST_GATHER:
            # w_arr[q, b*T+t] = idf[chunk[b, q*16+t]]
            w = idf[:, 0][chunk_c]                      # [BL, S]
            w = w.reshape(BL, P, T).transpose(1, 0, 2)  # [q, b, t]
            m["w_arr"] = np.ascontiguousarray(w.reshape(P, BL * T))
        in_maps.append(m)
    return in_maps


def kernel(chunk: np.ndarray, encoding: np.ndarray, idf: np.ndarray) -> np.ndarray:
    nc = _get_nc()
    in_maps = make_in_maps(chunk, encoding, idf)
    res = run_bass_kernel_spmd(nc, in_maps, core_ids=list(range(NCORES)))
    out = np.concatenate([res.results[c]["out"] for c in range(NCORES)], axis=0)
    return out.astype(np.float32)


if __name__ == "__main__":
    rng = np.random.default_rng(0)
    chunk = rng.integers(0, V, size=(B, S), dtype=np.int32)
    encoding = rng.standard_normal((B, S, D), dtype=np.float32)
    idf = rng.uniform(1e-3, 1.0, size=(V,)).astype(np.float32)
    out = kernel(chunk=chunk, encoding=encoding, idf=idf)
    print("out", out.shape, out.dtype, out[0, :4])



# revision 4
# speedup vs baseline: 4.8979x; 4.8979x over previous
"""DocumentDualEmbedder pooling kernel for Trainium2 (Bass/Tile).

Per doc b (B=64 docs, S=2048 tokens, D=256 dims):
    w     = idf[chunk[b]];  wn = w / sum(w)
    out[b] = concat(sum_s wn[s]*x[s],            # idf-weighted mean  [D]
                    max_s x[s], min_s x[s],      # max / min pool     [D each]
                    sqrt(S/(S-1)*(E[x^2]-mu^2))) # unbiased std       [D]

Distribution: pure data parallel over the batch dim -- each of the 8
NeuronCores processes 8 docs, no collectives.  Host prep (inside
kernel()): bf16 cast + device-layout swizzle of encoding, idf gather +
normalization into a packed [1/S | w_norm] stationary table.

Device structure per core (s = q*16 + t, q = partition, t = chunk):
  - docs stream in pairs: per-doc 1MB HWDGE loads (8KB/partition
    contiguous), per-doc ACT squares into a separate tile so the
    x-stream matmuls and the max/min trees depend only on the DMA.
  - mean/mu: 16 matmuls per doc, lhsT = [1/S | w_norm] (bf16), rhs = x_t;
    E[x^2]: 16 matmuls per doc, lhsT = [1/S], rhs = sq_t.  Both accumulate
    in per-half PSUM tiles that the tail reads directly (no per-doc
    drains; all PSUM engine reads are partition-base 0).
  - max/min: DVE pairwise trees in bf16 (2x mode), fused across the doc
    pair; partition reduction per half via TAIL_MODE:
      "transpose": PE transpose (via identity) -> PSUM -> DVE free-dim
                   reduce -> PE transpose back -> ACT drain -> out DMA.
      "gpsimd":    gpsimd partition_all_reduce(max) (min via negate).
  - std: musq = Square(mu) on ACT, var = E - musq on DVE, sqrt on ACT.
  - Small/output DMAs ride the SP HWDGE ring; tails run at high
    scheduler priority so the first half's epilogue overlaps the second
    half's main loop.
"""

import numpy as np
import ml_dtypes

import concourse.bass as bass
import concourse.bacc as bacc
import concourse.tile as tile
from concourse import mybir, bass_isa
from concourse.bass_utils import run_bass_kernel_spmd

B, S, D, V = 64, 2048, 256, 32000
NCORES = 8
BL = B // NCORES          # 8 docs per core
T = 16                    # chunks per doc
P = 128                   # partitions
HB = BL // 2              # half-batch (4 docs)
NP = BL // 2              # doc pairs
F32 = mybir.dt.float32
BF16 = mybir.dt.bfloat16
STD_SCALE = float(S) / float(S - 1)

TAIL_MODE = "transpose"


def build_bass(reps: int = 1):
    nc = bacc.Bacc("TRN2", target_bir_lowering=False, debug=False)
    xarr_d = nc.dram_tensor("xarr", [P, BL * T * D], BF16, kind="ExternalInput")
    wl_d = nc.dram_tensor("wl", [P, BL * T * 2], BF16, kind="ExternalInput")
    ident_d = nc.dram_tensor("ident", [P, P], BF16, kind="ExternalInput")
    out_d = nc.dram_tensor("out", [BL, 4 * D], F32, kind="ExternalOutput")

    with tile.TileContext(nc) as tc:
      for _rep in range(reps):
        with (
            tc.tile_pool(name="singles", bufs=1) as singles,
            tc.tile_pool(name="xpool", bufs=3) as xpool,
            tc.tile_pool(name="treepool", bufs=2) as treepool,
            tc.tile_pool(name="tailpool", bufs=1) as tailpool,
            tc.tile_pool(name="pstat", bufs=1, space="PSUM") as pstat,
            tc.tile_pool(name="ptrans", bufs=1, space="PSUM") as ptrans,
            tc.tile_pool(name="prps", bufs=1, space="PSUM") as prps,
        ):
            wl = singles.tile([P, BL, T, 2], BF16)
            nc.sync.dma_start(out=wl[:], in_=wl_d[:, :])
            ident = None
            if TAIL_MODE == "transpose":
                ident = singles.tile([P, P], BF16)
                nc.sync.dma_start(out=ident[:], in_=ident_d[:, :])

            mall = [singles.tile([P, HB, D], BF16, name=f"mall{h}", tag=f"mall{h}")
                    for h in range(2)]
            nall = [singles.tile([P, HB, D], BF16, name=f"nall{h}", tag=f"nall{h}")
                    for h in range(2)]

            def tail_stats(h, ps, psB, b0):
                # ps row 0 = mu, row 1 = mean (all PSUM reads base-0)
                mr2 = tailpool.tile([2, HB, D], F32, tag="mr2")
                nc.scalar.copy(mr2[:], ps[:, :, :])
                nc.sync.dma_start(out=out_d[b0:b0 + HB, 0:D], in_=mr2[1:2, :, :])
                musq = tailpool.tile([1, HB, D], F32, tag="musq")
                nc.scalar.activation(
                    musq[:], mr2[0:1, :, :],
                    mybir.ActivationFunctionType.Square)
                var0 = tailpool.tile([1, HB, D], F32, tag="var0")
                nc.vector.tensor_tensor(
                    var0[:], psB[0:1, :, :], musq[:],
                    op=mybir.AluOpType.subtract)
                stdrow = tailpool.tile([1, HB, D], F32, tag="stdrow")
                nc.scalar.activation(
                    stdrow[:], var0[:], mybir.ActivationFunctionType.Sqrt,
                    scale=STD_SCALE)
                nc.sync.dma_start(
                    out=out_d[b0:b0 + HB, 3 * D:4 * D], in_=stdrow[:])

            def tail_half_transpose(h, ps, psB):
                b0 = h * HB
                for stat, acc, alu in (("mx", mall[h], mybir.AluOpType.max),
                                       ("mn", nall[h], mybir.AluOpType.min)):
                    trp = ptrans.tile([P, 2 * HB, P], BF16, tag=f"trp{stat}")
                    for j in range(HB):
                        for k in range(2):
                            nc.tensor.transpose(
                                trp[:, 2 * j + k, :],
                                acc[:, j, k * P:(k + 1) * P],
                                ident[:],
                            )
                    red = tailpool.tile([P, 2 * HB], BF16, tag=f"red{stat}")
                    nc.vector.tensor_reduce(
                        red[:], trp[:], axis=mybir.AxisListType.X, op=alu)
                    rps = prps.tile([2 * HB, P], BF16, tag="rps")
                    nc.tensor.transpose(rps[:], red[:], ident[:])
                    rsb = tailpool.tile([2 * HB, P], F32, tag=f"rsb{stat}")
                    nc.scalar.copy(rsb[:], rps[:])
                    col = D if stat == "mx" else 2 * D
                    nc.sync.dma_start(
                        out=out_d[b0:b0 + HB, col:col + D], in_=rsb[:])
                tail_stats(h, ps, psB, b0)

            def tail_half_gpsimd(h, ps, psB):
                b0 = h * HB
                nneg = tailpool.tile([P, HB, D], BF16, tag="nneg")
                nc.vector.tensor_scalar_mul(nneg[:], nall[h][:], -1.0)
                mred = tailpool.tile([P, HB, D], F32, tag="mred")
                nc.gpsimd.partition_all_reduce(
                    mred[:], mall[h][:], channels=P,
                    reduce_op=bass_isa.ReduceOp.max)
                nc.sync.dma_start(
                    out=out_d[b0:b0 + HB, D:2 * D], in_=mred[0:1, :, :])
                nred = tailpool.tile([P, HB, D], F32, tag="nred")
                nc.gpsimd.partition_all_reduce(
                    nred[:], nneg[:], channels=P,
                    reduce_op=bass_isa.ReduceOp.max)
                minrow = tailpool.tile([1, HB, D], F32, tag="minrow")
                nc.scalar.mul(minrow[:], nred[0:1, :, :], -1.0)
                nc.sync.dma_start(
                    out=out_d[b0:b0 + HB, 2 * D:3 * D], in_=minrow[:])
                tail_stats(h, ps, psB, b0)

            tail_half = (tail_half_gpsimd if TAIL_MODE == "gpsimd"
                         else tail_half_transpose)

            ps_tiles = {}
            psB_tiles = {}
            for p in range(NP):
                h, pj = divmod(p, NP // 2)      # half, pair-within-half
                b0 = 2 * p
                # x and sq in separate tiles: the x-stream matmuls and the
                # trees depend only on the DMA, not on the squares.
                xt = xpool.tile([P, 2, T, D], BF16, tag="xt")
                sq = xpool.tile([P, 2, T, D], BF16, tag="sqt")
                for dj in range(2):
                    nc.sync.dma_start(
                        out=xt[:, dj, :, :],
                        in_=xarr_d[:, (b0 + dj) * T * D:(b0 + dj + 1) * T * D]
                        .rearrange("q (t d) -> q t d", d=D))
                    nc.scalar.activation(
                        sq[:, dj, :, :], xt[:, dj, :, :],
                        mybir.ActivationFunctionType.Square)

                if pj == 0:
                    ps_tiles[h] = pstat.tile([2, HB, D], F32, name="ps", tag="ps")
                    psB_tiles[h] = pstat.tile([1, HB, D], F32, name="psB", tag="psB")
                ps = ps_tiles[h]
                psB = psB_tiles[h]
                for dj in range(2):
                    b = b0 + dj
                    jj = b - h * HB
                    for t in range(T):
                        nc.tensor.matmul(
                            ps[:, jj, :],
                            lhsT=wl[:, b, t, :],
                            rhs=xt[:, dj, t, :],
                            start=(t == 0),
                            stop=(t == T - 1),
                            skip_group_check=True,
                        )
                    for t in range(T):
                        nc.tensor.matmul(
                            psB[:, jj, :],
                            lhsT=wl[:, b, t, 0:1],
                            rhs=sq[:, dj, t, :],
                            start=(t == 0),
                            stop=(t == T - 1),
                            skip_group_check=True,
                        )

                x_p = xt[:, :, :, :]
                jj0 = b0 - h * HB
                for stat, alu, acc in (("mx", mybir.AluOpType.max, mall[h]),
                                       ("mn", mybir.AluOpType.min, nall[h])):
                    t1 = treepool.tile([P, 2, 8, D], BF16, tag=f"{stat}1")
                    nc.vector.tensor_tensor(
                        t1[:], x_p[:, :, 0:8, :], x_p[:, :, 8:16, :], op=alu)
                    t2 = treepool.tile([P, 2, 4, D], BF16, tag=f"{stat}2")
                    nc.vector.tensor_tensor(
                        t2[:], t1[:, :, 0:4, :], t1[:, :, 4:8, :], op=alu)
                    t3 = treepool.tile([P, 2, 2, D], BF16, tag=f"{stat}3")
                    nc.vector.tensor_tensor(
                        t3[:], t2[:, :, 0:2, :], t2[:, :, 2:4, :], op=alu)
                    nc.vector.tensor_tensor(
                        acc[:, jj0:jj0 + 2, :], t3[:, :, 0, :], t3[:, :, 1, :],
                        op=alu)

                if pj == NP // 2 - 1:
                    with tc.high_priority():
                        tail_half(h, ps_tiles[h], psB_tiles[h])

    nc.finalize()
    return nc


_NC = None


def _get_nc():
    global _NC
    if _NC is None:
        _NC = build_bass()
    return _NC


def make_in_maps(chunk, encoding, idf):
    chunk = np.ascontiguousarray(np.asarray(chunk, dtype=np.int32))
    encoding = np.asarray(encoding, dtype=np.float32)
    idf = np.asarray(idf, dtype=np.float32).reshape(V)
    ident = np.eye(P, dtype=ml_dtypes.bfloat16)
    in_maps = []
    for c in range(NCORES):
        sl = slice(c * BL, (c + 1) * BL)
        # [b, s, d] -> [q, b, t, d], bf16
        xa = encoding[sl].reshape(BL, P, T, D).transpose(1, 0, 2, 3)
        xa = np.ascontiguousarray(xa).astype(ml_dtypes.bfloat16)
        w = idf[chunk[sl]]                          # [BL, S]
        w = w / w.sum(axis=1, keepdims=True)
        wl = np.empty((P, BL, T, 2), dtype=np.float32)
        wl[..., 0] = 1.0 / S
        wl[..., 1] = w.reshape(BL, P, T).transpose(1, 0, 2)
        in_maps.append({
            "xarr": xa.reshape(P, BL * T * D),
            "wl": wl.reshape(P, BL * T * 2).astype(ml_dtypes.bfloat16),
            "ident": ident,
        })
    return in_maps


def kernel(chunk: np.ndarray, encoding: np.ndarray, idf: np.ndarray) -> np.ndarray:
    nc = _get_nc()
    in_maps = make_in_maps(chunk, encoding, idf)
    res = run_bass_kernel_spmd(nc, in_maps, core_ids=list(range(NCORES)))
    out = np.concatenate([res.results[c]["out"] for c in range(NCORES)], axis=0)
    return out.astype(np.float32)


if __name__ == "__main__":
    rng = np.random.default_rng(0)
    chunk = rng.integers(0, V, size=(B, S), dtype=np.int32)
    encoding = rng.standard_normal((B, S, D), dtype=np.float32)
    idf = rng.uniform(1e-3, 1.0, size=(V,)).astype(np.float32)
    out = kernel(chunk=chunk, encoding=encoding, idf=idf)
    print("out", out.shape, out.dtype, out[0, :4])


# revision 6
# speedup vs baseline: 6.8523x; 1.3990x over previous
"""DocumentDualEmbedder pooling kernel for Trainium2 (Bass/Tile).

Per doc b (B=64 docs, S=2048 tokens, D=256 dims):
    w     = idf[chunk[b]];  wn = w / sum(w)
    out[b] = concat(sum_s wn[s]*x[s],            # idf-weighted mean  [D]
                    max_s x[s], min_s x[s],      # max / min pool     [D each]
                    sqrt(S/(S-1)*(E[x^2]-mu^2))) # unbiased std       [D]

Distribution: pure data parallel over the batch dim -- each of the 8
NeuronCores processes 8 docs, no collectives.  Host prep (inside
kernel()): bf16 cast + device-layout swizzle of encoding, idf gather +
normalization into a packed [1/S | w_norm] stationary table.

Device structure per core (s = q*16 + t, q = partition, t = chunk):
  - docs stream in pairs: per-doc 1MB HWDGE loads (8KB/partition
    contiguous), per-doc ACT squares into a separate tile so the
    x-stream matmuls and the max/min trees depend only on the DMA.
  - mean/mu: 16 matmuls per doc, lhsT = [1/S | w_norm] (bf16), rhs = x_t;
    E[x^2]: 16 matmuls per doc, lhsT = [1/S], rhs = sq_t.  Both accumulate
    in per-half PSUM tiles that the tail reads directly (no per-doc
    drains; all PSUM engine reads are partition-base 0).
  - max/min: DVE pairwise trees in bf16 (2x mode), fused across the doc
    pair; partition reduction per half via TAIL_MODE:
      "transpose": PE transpose (via identity) -> PSUM -> DVE free-dim
                   reduce -> PE transpose back -> ACT drain -> out DMA.
      "gpsimd":    gpsimd partition_all_reduce(max) (min via negate).
  - std: musq = Square(mu) on ACT, var = E - musq on DVE, sqrt on ACT.
  - Small/output DMAs ride the SP HWDGE ring; tails run at high
    scheduler priority so the first half's epilogue overlaps the second
    half's main loop.
"""

import numpy as np
import ml_dtypes

import concourse.bass as bass
import concourse.bacc as bacc
import concourse.tile as tile
from concourse import mybir, bass_isa
from concourse.bass_utils import run_bass_kernel_spmd

B, S, D, V = 64, 2048, 256, 32000
NCORES = 8
BL = B // NCORES          # 8 docs per core
T = 16                    # chunks per doc
P = 128                   # partitions
HB = BL // 2              # half-batch (4 docs)
NP = BL // 2              # doc pairs
F32 = mybir.dt.float32
BF16 = mybir.dt.bfloat16
STD_SCALE = float(S) / float(S - 1)

TAIL_MODE = "transpose"


def build_bass(reps: int = 1):
    nc = bacc.Bacc("TRN2", target_bir_lowering=False, debug=False)
    xarr_d = nc.dram_tensor("xarr", [P, BL * T * D], BF16, kind="ExternalInput")
    wl_d = nc.dram_tensor("wl", [P, BL * T * 2], BF16, kind="ExternalInput")
    ident_d = nc.dram_tensor("ident", [P, P], BF16, kind="ExternalInput")
    out_d = nc.dram_tensor("out", [BL, 4 * D], F32, kind="ExternalOutput")

    with tile.TileContext(nc) as tc:
      for _rep in range(reps):
        with (
            tc.tile_pool(name="singles", bufs=1) as singles,
            tc.tile_pool(name="xpool", bufs=3) as xpool,
            tc.tile_pool(name="treepool", bufs=2) as treepool,
            tc.tile_pool(name="tailpool", bufs=1) as tailpool,
            tc.tile_pool(name="pstat", bufs=1, space="PSUM") as pstat,
            tc.tile_pool(name="ptrans", bufs=1, space="PSUM") as ptrans,
            tc.tile_pool(name="prps", bufs=1, space="PSUM") as prps,
        ):
            wl = singles.tile([P, BL, T, 2], BF16)
            nc.scalar.dma_start(out=wl[:], in_=wl_d[:, :])
            ident = None
            if TAIL_MODE == "transpose":
                ident = singles.tile([P, P], BF16)
                nc.scalar.dma_start(out=ident[:], in_=ident_d[:, :])

            mall = [singles.tile([P, HB, D], BF16, name=f"mall{h}", tag=f"mall{h}")
                    for h in range(2)]
            nall = [singles.tile([P, HB, D], BF16, name=f"nall{h}", tag=f"nall{h}")
                    for h in range(2)]

            def tail_stats(h, ps, psB, b0):
                # ps row 0 = mu, row 1 = mean (all PSUM reads base-0)
                musq = tailpool.tile([1, HB, D], F32, tag="musq")
                nc.scalar.activation(
                    musq[:], ps[0:1, :, :],
                    mybir.ActivationFunctionType.Square)
                mr2 = tailpool.tile([2, HB, D], F32, tag="mr2")
                nc.scalar.copy(mr2[:], ps[:, :, :])
                nc.sync.dma_start(out=out_d[b0:b0 + HB, 0:D], in_=mr2[1:2, :, :])
                var0 = tailpool.tile([1, HB, D], F32, tag="var0")
                nc.vector.tensor_tensor(
                    var0[:], psB[0:1, :, :], musq[:],
                    op=mybir.AluOpType.subtract)
                stdrow = tailpool.tile([1, HB, D], F32, tag="stdrow")
                nc.scalar.activation(
                    stdrow[:], var0[:], mybir.ActivationFunctionType.Sqrt,
                    scale=STD_SCALE)
                nc.sync.dma_start(
                    out=out_d[b0:b0 + HB, 3 * D:4 * D], in_=stdrow[:])

            def tail_half_transpose(h, ps, psB):
                b0 = h * HB
                for stat, acc, alu in (("mx", mall[h], mybir.AluOpType.max),
                                       ("mn", nall[h], mybir.AluOpType.min)):
                    trp = ptrans.tile([P, 2 * HB, P], BF16, tag=f"trp{stat}")
                    for j in range(HB):
                        for k in range(2):
                            nc.tensor.transpose(
                                trp[:, 2 * j + k, :],
                                acc[:, j, k * P:(k + 1) * P],
                                ident[:],
                            )
                    red = tailpool.tile([P, 2 * HB], BF16, tag=f"red{stat}")
                    nc.vector.tensor_reduce(
                        red[:], trp[:], axis=mybir.AxisListType.X, op=alu)
                    rps = prps.tile([2 * HB, P], BF16, tag="rps")
                    nc.tensor.transpose(rps[:], red[:], ident[:])
                    rsb = tailpool.tile([2 * HB, P], F32, tag=f"rsb{stat}")
                    nc.scalar.copy(rsb[:], rps[:])
                    col = D if stat == "mx" else 2 * D
                    nc.sync.dma_start(
                        out=out_d[b0:b0 + HB, col:col + D], in_=rsb[:])
                tail_stats(h, ps, psB, b0)

            def tail_half_gpsimd(h, ps, psB):
                b0 = h * HB
                nneg = tailpool.tile([P, HB, D], BF16, tag="nneg")
                nc.vector.tensor_scalar_mul(nneg[:], nall[h][:], -1.0)
                mred = tailpool.tile([P, HB, D], F32, tag="mred")
                nc.gpsimd.partition_all_reduce(
                    mred[:], mall[h][:], channels=P,
                    reduce_op=bass_isa.ReduceOp.max)
                nc.sync.dma_start(
                    out=out_d[b0:b0 + HB, D:2 * D], in_=mred[0:1, :, :])
                nred = tailpool.tile([P, HB, D], F32, tag="nred")
                nc.gpsimd.partition_all_reduce(
                    nred[:], nneg[:], channels=P,
                    reduce_op=bass_isa.ReduceOp.max)
                minrow = tailpool.tile([1, HB, D], F32, tag="minrow")
                nc.scalar.mul(minrow[:], nred[0:1, :, :], -1.0)
                nc.sync.dma_start(
                    out=out_d[b0:b0 + HB, 2 * D:3 * D], in_=minrow[:])
                tail_stats(h, ps, psB, b0)

            tail_half = (tail_half_gpsimd if TAIL_MODE == "gpsimd"
                         else tail_half_transpose)

            ps_tiles = {}
            psB_tiles = {}
            for p in range(NP):
                h, pj = divmod(p, NP // 2)      # half, pair-within-half
                b0 = 2 * p
                # x and sq in separate tiles: the x-stream matmuls and the
                # trees depend only on the DMA, not on the squares.
                xt = xpool.tile([P, 2, T, D], BF16, tag="xt")
                sq = xpool.tile([P, 2, T, D], BF16, tag="sqt")
                for dj in range(2):
                    nc.sync.dma_start(
                        out=xt[:, dj, :, :],
                        in_=xarr_d[:, (b0 + dj) * T * D:(b0 + dj + 1) * T * D]
                        .rearrange("q (t d) -> q t d", d=D))
                    nc.scalar.activation(
                        sq[:, dj, :, :], xt[:, dj, :, :],
                        mybir.ActivationFunctionType.Square)

                if pj == 0:
                    ps_tiles[h] = pstat.tile([2, HB, D], F32, name="ps", tag="ps")
                    psB_tiles[h] = pstat.tile([1, HB, D], F32, name="psB", tag="psB")
                ps = ps_tiles[h]
                psB = psB_tiles[h]
                for dj in range(2):
                    b = b0 + dj
                    jj = b - h * HB
                    for t in range(T):
                        nc.tensor.matmul(
                            ps[:, jj, :],
                            lhsT=wl[:, b, t, :],
                            rhs=xt[:, dj, t, :],
                            start=(t == 0),
                            stop=(t == T - 1),
                            skip_group_check=True,
                        )
                    for t in range(T):
                        nc.tensor.matmul(
                            psB[:, jj, :],
                            lhsT=wl[:, b, t, 0:1],
                            rhs=sq[:, dj, t, :],
                            start=(t == 0),
                            stop=(t == T - 1),
                            skip_group_check=True,
                        )

                if p == 0:
                    # first pair: per-doc trees so the DVE starts as soon
                    # as doc 0 lands (it back-pressures the whole kernel)
                    for dj in range(2):
                        x_d = xt[:, dj, :, :]
                        jj = b0 + dj - h * HB
                        for stat, alu, acc in (("mx", mybir.AluOpType.max, mall[h]),
                                               ("mn", mybir.AluOpType.min, nall[h])):
                            t1 = treepool.tile([P, 8, D], BF16, name="t1", tag=f"{stat}1")
                            nc.vector.tensor_tensor(
                                t1[:], x_d[:, 0:8, :], x_d[:, 8:16, :], op=alu)
                            t2 = treepool.tile([P, 4, D], BF16, name="t2", tag=f"{stat}2")
                            nc.vector.tensor_tensor(
                                t2[:], t1[:, 0:4, :], t1[:, 4:8, :], op=alu)
                            t3 = treepool.tile([P, 2, D], BF16, name="t3", tag=f"{stat}3")
                            nc.vector.tensor_tensor(
                                t3[:], t2[:, 0:2, :], t2[:, 2:4, :], op=alu)
                            nc.vector.tensor_tensor(
                                acc[:, jj, :], t3[:, 0, :], t3[:, 1, :], op=alu)
                else:
                    # later pairs: fused across the pair (fewer op inits)
                    x_p = xt[:, :, :, :]
                    jj0 = b0 - h * HB
                    for stat, alu, acc in (("mx", mybir.AluOpType.max, mall[h]),
                                           ("mn", mybir.AluOpType.min, nall[h])):
                        p1 = treepool.tile([P, 2, 8, D], BF16, name="p1", tag=f"p{stat}1")
                        nc.vector.tensor_tensor(
                            p1[:], x_p[:, :, 0:8, :], x_p[:, :, 8:16, :], op=alu)
                        p2 = treepool.tile([P, 2, 4, D], BF16, name="p2", tag=f"p{stat}2")
                        nc.vector.tensor_tensor(
                            p2[:], p1[:, :, 0:4, :], p1[:, :, 4:8, :], op=alu)
                        p3 = treepool.tile([P, 2, 2, D], BF16, name="p3", tag=f"p{stat}3")
                        nc.vector.tensor_tensor(
                            p3[:], p2[:, :, 0:2, :], p2[:, :, 2:4, :], op=alu)
                        nc.vector.tensor_tensor(
                            acc[:, jj0:jj0 + 2, :], p3[:, :, 0, :], p3[:, :, 1, :],
                            op=alu)

                if pj == NP // 2 - 1:
                    with tc.high_priority():
                        tail_half(h, ps_tiles[h], psB_tiles[h])

    nc.finalize()
    return nc


_NC = None


def _get_nc():
    global _NC
    if _NC is None:
        _NC = build_bass()
    return _NC


def make_in_maps(chunk, encoding, idf):
    chunk = np.ascontiguousarray(np.asarray(chunk, dtype=np.int32))
    encoding = np.asarray(encoding, dtype=np.float32)
    idf = np.asarray(idf, dtype=np.float32).reshape(V)
    ident = np.eye(P, dtype=ml_dtypes.bfloat16)
    in_maps = []
    for c in range(NCORES):
        sl = slice(c * BL, (c + 1) * BL)
        # [b, s, d] -> [q, b, t, d], bf16
        xa = encoding[sl].reshape(BL, P, T, D).transpose(1, 0, 2, 3)
        xa = np.ascontiguousarray(xa).astype(ml_dtypes.bfloat16)
        w = idf[chunk[sl]]                          # [BL, S]
        w = w / w.sum(axis=1, keepdims=True)
        wl = np.empty((P, BL, T, 2), dtype=np.float32)
        wl[..., 0] = 1.0 / S
        wl[..., 1] = w.reshape(BL, P, T).transpose(1, 0, 2)
        in_maps.append({
            "xarr": xa.reshape(P, BL * T * D),
            "wl": wl.reshape(P, BL * T * 2).astype(ml_dtypes.bfloat16),
            "ident": ident,
        })
    return in_maps


def kernel(chunk: np.ndarray, encoding: np.ndarray, idf: np.ndarray) -> np.ndarray:
    nc = _get_nc()
    in_maps = make_in_maps(chunk, encoding, idf)
    res = run_bass_kernel_spmd(nc, in_maps, core_ids=list(range(NCORES)))
    out = np.concatenate([res.results[c]["out"] for c in range(NCORES)], axis=0)
    return out.astype(np.float32)


if __name__ == "__main__":
    rng = np.random.default_rng(0)
    chunk = rng.integers(0, V, size=(B, S), dtype=np.int32)
    encoding = rng.standard_normal((B, S, D), dtype=np.float32)
    idf = rng.uniform(1e-3, 1.0, size=(V,)).astype(np.float32)
    out = kernel(chunk=chunk, encoding=encoding, idf=idf)
    print("out", out.shape, out.dtype, out[0, :4])
